# revision 42
# baseline (speedup 1.0000x reference)
"""NaViT packed-sequence ViT forward on 8 Trainium2 NeuronCores.

Sharding: the packed groups have block-diagonal attention (4 images x 256
tokens per group, verified at runtime from image_ids/lengths). The network
then decomposes per image -> 16 independent images, 2 per core, T=512
tokens per core, zero cross-core communication.

Per-core kernel layout:
  - residual stream x: token-major [128 part, 4 tiles, 768] fp32
  - matmul operands bf16; LN/softmax/statistics fp32
  - W2 of the MLP runs fp8e4 DoubleRow (2x PE throughput) with h produced
    in fp8 straight out of the GELU eviction; W1 stays bf16 (activation-
    quantization error budget, and W1's GELU eviction would pace a DR W1)
  - the qk gammas fold into the q side so k needs only its own rms scale
  - attention per (image, head) with scores^T [j, i]; softmax denominator
    via an extra ones-column appended to V; no max-subtraction needed
  - LN gammas folded into the consuming weight matrices host-side
  - emission is stage-wavefront and cross-stage staggered so PE never
    waits on DVE LN chains (W2(t+1) covers LN(t), next-layer QKV covers
    the tail LNs, pool-KV GEMMs cover the final LNs)
"""

import sys

sys.path.insert(0, "/opt/trn_rl_repo")

import numpy as np
import ml_dtypes

B, IMGS, PH, PW = 4, 4, 16, 16
P, C = 16, 3
N = IMGS * PH * PW            # 1024
PATCH_DIM = C * P * P         # 768
DIM, HEADS, DHEAD, DEPTH = 768, 12, 64, 4
INNER = HEADS * DHEAD         # 768
MLP = 3072
NCLS = 1000
SCALE = DHEAD ** 0.5

NCORES = 8
IMG_PER_CORE = 2
T = IMG_PER_CORE * PH * PW    # 512 tokens per core
TT = T // 128                 # 4 token tiles
KD = DIM // 128               # 6 feature chunks
MC = MLP // 128               # 24 mlp chunks
TOK_IMG = PH * PW             # 256

W_SCALE = 2048.0              # power-2 weight scale into fp8 range
W1_DEQ = 1.0 / W_SCALE


def _ln_np(x, g, eps=1e-5):
    mu = x.mean(-1, keepdims=True)
    var = x.var(-1, keepdims=True)
    return (x - mu) / np.sqrt(var + eps) * g


def _rms_np(t, g):
    nrm = np.sqrt((t * t).sum(-1, keepdims=True))
    return t / np.maximum(nrm, 1e-12) * SCALE * g


def _softmax_np(x, axis):
    m = x.max(axis=axis, keepdims=True)
    e = np.exp(x - m)
    return e / e.sum(axis=axis, keepdims=True)


def _erf(x):
    try:
        from scipy.special import erf as _serf
        return _serf(x)
    except Exception:
        import math
        return np.vectorize(math.erf)(x).astype(x.dtype)


def _reference_np(**inp):
    """Numpy port of the oracle; only used for non-block-diagonal inputs."""
    f32 = np.float32
    patches = inp["patches"].astype(f32)
    image_ids = np.asarray(inp["image_ids"])
    lengths = np.asarray(inp["lengths"])
    valid = np.arange(N)[None, :] < lengths[:, None]
    same = image_ids[:, :, None] == image_ids[:, None, :]
    attn_mask = (same & valid[:, None, :])[:, None]

    x = _ln_np(patches, inp["emb_ln_g"]) @ inp["W_emb"] + inp["b_emb"]
    x = _ln_np(x, inp["emb_ln2_g"])
    pp = np.asarray(inp["patch_positions"])
    x = x + inp["pos_h"][pp[..., 0]] + inp["pos_w"][pp[..., 1]]

    def attention(x_, context, ln_g, Wq, Wkv, qg, kg, Wo, mask):
        xn = _ln_np(x_, ln_g)
        kv_in = xn if context is None else context
        q = xn @ Wq
        k, v = np.split(kv_in @ Wkv, 2, axis=-1)

        def split(t):
            return t.reshape(t.shape[0], t.shape[1], HEADS, DHEAD).transpose(0, 2, 1, 3)

        q, k, v = split(q), split(k), split(v)
        q = _rms_np(q, qg[:, None, :])
        k = _rms_np(k, kg[:, None, :])
        dots = np.einsum("bhid,bhjd->bhij", q, k)
        dots = np.where(mask, dots, -np.finfo(f32).max)
        attn = _softmax_np(dots, -1)
        out = np.einsum("bhij,bhjd->bhid", attn, v)
        out = out.transpose(0, 2, 1, 3).reshape(x_.shape[0], -1, INNER)
        return out @ Wo

    for l in range(DEPTH):
        x = attention(x, None, inp["ln_attn_g"][l], inp["Wq"][l], inp["Wkv"][l],
                      inp["qn_g"][l], inp["kn_g"][l], inp["Wo"][l], attn_mask) + x
        h = _ln_np(x, inp["ln_ff_g"][l]) @ inp["W1"][l] + inp["b1"][l]
        h = h * 0.5 * (1.0 + _erf(h / np.sqrt(2.0)))
        x = h @ inp["W2"][l] + inp["b2"][l] + x
    x = _ln_np(x, inp["final_ln_g"])

    queries = np.broadcast_to(inp["pool_q"], (B, IMGS, DIM))
    pool_mask = ((np.arange(IMGS)[None, :, None] == image_ids[:, None, :])
                 & valid[:, None, :])[:, None]
    pooled = attention(queries, x, inp["pool_ln_g"], inp["pWq"], inp["pWkv"],
                       inp["p_qn_g"], inp["p_kn_g"], inp["pWo"], pool_mask) + queries
    return (_ln_np(pooled, inp["head_ln_g"]) @ inp["W_head"]).astype(f32)


# ---------------------------------------------------------------------------
# Bass kernel
# ---------------------------------------------------------------------------

_CACHE = {}


def build_kernel(cfg=None):
    cfg = cfg or {}
    import concourse.bass as bass
    from concourse import bacc
    import concourse.mybir as mybir
    import concourse.tile as tile
    from concourse.masks import make_identity

    F32 = mybir.dt.float32
    BF16 = mybir.dt.bfloat16
    W8 = mybir.dt.float8e4
    DR = mybir.MatmulPerfMode.DoubleRow
    AF = mybir.ActivationFunctionType
    ALU = mybir.AluOpType
    AX = mybir.AxisListType
    use_bemb = cfg.get("bemb", False)
    use_b2 = cfg.get("b2", False)

    nc = bacc.Bacc()

    def din(name, shape, dt=F32):
        return nc.declare_dram_parameter(name, list(shape), dt, isOutput=False)

    patches_d = din("patches", [T, PATCH_DIM])
    pos_d = din("pos_add", [T, DIM])
    W_emb_d = din("W_emb", [PATCH_DIM, DIM], BF16)
    Wq_d = din("Wq", [DEPTH, DIM, INNER], BF16)
    Wkv_d = din("Wkv", [DEPTH, DIM, 2 * INNER], BF16)
    Wo_d = din("Wo", [DEPTH, INNER, DIM], BF16)
    W1_d = din("W1", [DEPTH, DIM, MLP], BF16)
    W2_d = din("W2", [DEPTH, MLP, DIM], W8)
    pWkv_d = din("pWkv", [DIM, 2 * INNER], BF16)
    pWo_d = din("pWo", [INNER, DIM], BF16)
    W_head_d = din("W_head", [DIM, NCLS], BF16)
    emb_ln2_g_d = din("emb_ln2_g", [DIM])
    b_emb_d = din("b_emb", [DIM])
    kg_row_d = din("kg_row", [DEPTH, INNER], BF16)   # qn_g * kn_g * SCALE^2
    b1_d = din("b1", [DEPTH, MLP])
    b2_d = din("b2", [DEPTH, DIM])
    pk_row_d = din("pk_row", [INNER], BF16)          # p_kn_g * SCALE
    qpool_d = din("qpool", [128, HEADS], BF16)       # per-head halves, zero-padded
    pool_q_d = din("pool_q", [DIM])
    out_d = nc.declare_dram_parameter("out", [IMG_PER_CORE, NCLS], F32,
                                      isOutput=True)

    def bcast_ap(dram, row, width, parts=128):
        ap = dram.ap()
        off = ap.offset + (0 if row is None else row * width)
        return bass.AP(tensor=ap.tensor, offset=off, ap=[[0, parts], [1, width]])

    with tile.TileContext(nc) as tc:
        with (
            tc.tile_pool(name="const", bufs=1) as constp,
            tc.tile_pool(name="resid", bufs=1) as residp,
            tc.tile_pool(name="act", bufs=1) as actp,
            tc.tile_pool(name="wts", bufs=4) as wpool,
            tc.tile_pool(name="embd", bufs=2) as embp,
            tc.tile_pool(name="aux", bufs=2) as auxp,
            tc.tile_pool(name="small", bufs=3) as smallp,
            tc.tile_pool(name="attn", bufs=3) as attnp,
            tc.tile_pool(name="ps", bufs=2, space="PSUM") as psp,
        ):
            ident = constp.tile([128, 128], BF16)
            make_identity(nc, ident)
            eps_t = constp.tile([128, 1], F32)
            nc.vector.memset(eps_t, 1e-5)
            eps2_t = constp.tile([128, 1], F32)
            nc.vector.memset(eps2_t, 1e-30)

            x_t = residp.tile([128, TT, DIM], F32, tag="x")
            xn_t = residp.tile([128, TT, DIM], BF16, tag="xn")
            xnT_t = residp.tile([128, KD, T], BF16, tag="xnT")
            q_t = residp.tile([128, TT, INNER], BF16, tag="q")
            k_t = residp.tile([128, TT, INNER], BF16, tag="k")
            v_t = residp.tile([128, TT, HEADS, DHEAD + 1], BF16, tag="v")
            qT_t = residp.tile([128, KD, T], BF16, tag="qT")
            kT_t = residp.tile([128, KD, T], BF16, tag="kT")
            av_t = q_t      # q is dead once qT exists; reuse for attn out
            avT_t = qT_t    # per-image columns of qT are dead after scores
            h_t = residp.tile([128, MC, T], W8, tag="h")

            nc.vector.memset(v_t[:, :, :, DHEAD:DHEAD + 1], 1.0)

            # ---------------- helpers ----------------
            def ln_to(dst, src_f32, its, gamma_bc=None, dst_it=None):
                """dst[:, dst_it or it] = LN(src[:, it]) [* gamma]."""
                for it in its:
                    ot = it if dst_it is None else dst_it
                    st = smallp.tile([128, 2, nc.vector.BN_STATS_DIM], F32,
                                     tag="st")
                    xr = src_f32[:, it, :].rearrange("p (n f) -> p n f", f=384)
                    for i in range(2):
                        nc.vector.bn_stats(out=st[:, i, :], in_=xr[:, i, :])
                    mv = smallp.tile([128, nc.vector.BN_AGGR_DIM], F32, tag="mv")
                    nc.vector.bn_aggr(out=mv, in_=st)
                    rstd = smallp.tile([128, 1], F32, tag="rstd")
                    nc.scalar.activation(out=rstd, in_=mv[:, 1:2], func=AF.Sqrt,
                                         bias=eps_t, scale=1.0)
                    nc.vector.reciprocal(out=rstd, in_=rstd)
                    if gamma_bc is None:
                        nc.vector.tensor_scalar(
                            out=dst[:, ot, :], in0=src_f32[:, it, :],
                            scalar1=mv[:, 0:1], scalar2=rstd,
                            op0=ALU.subtract, op1=ALU.mult)
                    else:
                        tmp = smallp.tile([128, DIM], F32, tag="lntmp",
                                          bufs=1)
                        nc.vector.scalar_tensor_tensor(
                            out=tmp, in0=src_f32[:, it, :], scalar=mv[:, 0:1],
                            in1=gamma_bc, op0=ALU.subtract, op1=ALU.mult)
                        nc.vector.tensor_scalar_mul(out=dst[:, ot, :], in0=tmp,
                                                    scalar1=rstd)

            def transpose_tm(dst_fm, src_tm, its, ncol=DIM):
                """token-major [128,TT,ncol] -> feature-major [128,nc,T]."""
                nch = ncol // 128
                for it in its:
                    ps = psp.tile([128, T + 256], BF16, tag="tp")
                    for c in range(nch):
                        nc.tensor.transpose(
                            ps[:, c * 128:(c + 1) * 128],
                            src_tm[:, it, c * 128:(c + 1) * 128], ident)
                    ps3 = ps[:, 0:nch * 128].rearrange("p (c w) -> p c w", w=128)
                    dst = dst_fm[:, :, it * 128:(it + 1) * 128]
                    nc.vector.tensor_copy(out=dst, in_=ps3)

            def load_w(dram_l, rows, cols, tag="wt", row_off=0, col_off=0,
                       dt=BF16):
                wt = wpool.tile([128, rows // 128, cols], dt, tag=tag)
                src = dram_l.rearrange("(c p) n -> p c n", p=128)
                nc.sync.dma_start(
                    out=wt, in_=src[:, row_off // 128:(row_off + rows) // 128,
                                    col_off:col_off + cols])
                return wt

            def mm_tok(dst_tm, lhsT_fm, w_tile, ncol, its, col_base=0,
                       add_resid=None, nk=KD):
                """token-major matmul: dst[:, it, g] = lhsT_fm.T @ w[:, col_base+g]."""
                for it in its:
                    for g0 in range(0, ncol, 512):
                        gw = min(512, ncol - g0)
                        ps = psp.tile([128, 512], F32, tag="mm")
                        for c in range(nk):
                            nc.tensor.matmul(
                                ps[:, :gw],
                                lhsT_fm[:, c, it * 128:(it + 1) * 128],
                                w_tile[:, c, col_base + g0:col_base + g0 + gw],
                                start=(c == 0), stop=(c == nk - 1))
                        if add_resid is not None:
                            nc.vector.tensor_add(
                                out=add_resid[:, it, g0:g0 + gw],
                                in0=add_resid[:, it, g0:g0 + gw], in1=ps[:, :gw])
                        else:
                            nc.scalar.activation(out=dst_tm[:, it, g0:g0 + gw],
                                                 in_=ps[:, :gw], func=AF.Copy)

            def mm_v(lhsT_fm, w_tile, col_base, its):
                """like mm_tok but scatters per-head into v_t's 65-stride slots
                with one batched strided eviction per psum chunk."""
                for it in its:
                    for g0 in range(0, INNER, 512):
                        gw = min(512, INNER - g0)
                        ps = psp.tile([128, 512], F32, tag="mm")
                        for c in range(KD):
                            nc.tensor.matmul(
                                ps[:, :gw],
                                lhsT_fm[:, c, it * 128:(it + 1) * 128],
                                w_tile[:, c, col_base + g0:col_base + g0 + gw],
                                start=(c == 0), stop=(c == KD - 1))
                        h0, nh = g0 // DHEAD, gw // DHEAD
                        nc.scalar.activation(
                            out=v_t[:, it, h0:h0 + nh, 0:DHEAD],
                            in_=ps[:, :gw].rearrange("p (h d) -> p h d", d=DHEAD),
                            func=AF.Copy)

            def rms_q(its, kg_bc):
                """q *= 1/||q_h|| per (token, head), then the folded qk gammas."""
                for it in its:
                    sq = smallp.tile([128, INNER], BF16, tag="sq", bufs=2)
                    nc.vector.tensor_mul(out=sq, in0=q_t[:, it, :],
                                         in1=q_t[:, it, :])
                    ss = smallp.tile([128, HEADS], BF16, tag="ss")
                    with nc.allow_low_precision(
                            reason="sum of 64 squares; DVE accumulates fp32 "
                                   "internally, only the output is bf16"):
                        nc.vector.tensor_reduce(
                            out=ss,
                            in_=sq.rearrange("p (h d) -> p h d", d=DHEAD),
                            axis=AX.X, op=ALU.add)
                    nrm = smallp.tile([128, HEADS], F32, tag="nrm")
                    nc.scalar.activation(out=nrm, in_=ss, func=AF.Sqrt,
                                         bias=eps2_t, scale=1.0)
                    nc.vector.reciprocal(out=nrm, in_=nrm)
                    nrb = smallp.tile([128, HEADS], BF16, tag="nrb")
                    nc.vector.tensor_copy(out=nrb, in_=nrm)
                    nr = nrb[:, :]
                    nr_bc = bass.AP(tensor=nr.tensor, offset=nr.offset,
                                    ap=[nr.ap[0], [1, HEADS], [0, DHEAD]])
                    t3 = q_t[:, it, :].rearrange("p (h d) -> p h d", d=DHEAD)
                    nc.vector.tensor_mul(out=t3, in0=t3, in1=nr_bc)
                    nc.vector.tensor_mul(out=q_t[:, it, :], in0=q_t[:, it, :],
                                         in1=kg_bc)

            def rms_knorm(its):
                """k *= 1/||k_h|| per (token, head); the qk gammas live on q."""
                for it in its:
                    sq = smallp.tile([128, INNER], BF16, tag="sq", bufs=2)
                    nc.vector.tensor_mul(out=sq, in0=k_t[:, it, :],
                                         in1=k_t[:, it, :])
                    ss = smallp.tile([128, HEADS], BF16, tag="ss")
                    with nc.allow_low_precision(
                            reason="sum of 64 squares; DVE accumulates fp32 "
                                   "internally, only the output is bf16"):
                        nc.vector.tensor_reduce(
                            out=ss,
                            in_=sq.rearrange("p (h d) -> p h d", d=DHEAD),
                            axis=AX.X, op=ALU.add)
                    nrm = smallp.tile([128, HEADS], F32, tag="nrm")
                    nc.scalar.activation(out=nrm, in_=ss, func=AF.Sqrt,
                                         bias=eps2_t, scale=1.0)
                    nc.vector.reciprocal(out=nrm, in_=nrm)
                    nrb = smallp.tile([128, HEADS], BF16, tag="nrb")
                    nc.vector.tensor_copy(out=nrb, in_=nrm)
                    nr = nrb[:, :]
                    nr_bc = bass.AP(tensor=nr.tensor, offset=nr.offset,
                                    ap=[nr.ap[0], [1, HEADS], [0, DHEAD]])
                    t3 = k_t[:, it, :].rearrange("p (h d) -> p h d", d=DHEAD)
                    nc.vector.tensor_mul(out=t3, in0=t3, in1=nr_bc)

            def rms_k_pool(its, kg_bc):
                """pool path: normalize k in place (old style) and apply pk."""
                for it in its:
                    sq = smallp.tile([128, INNER], BF16, tag="sq", bufs=2)
                    nc.vector.tensor_mul(out=sq, in0=k_t[:, it, :],
                                         in1=k_t[:, it, :])
                    ss = smallp.tile([128, HEADS], BF16, tag="ss")
                    with nc.allow_low_precision(
                            reason="sum of 64 squares; DVE accumulates fp32 "
                                   "internally, only the output is bf16"):
                        nc.vector.tensor_reduce(
                            out=ss,
                            in_=sq.rearrange("p (h d) -> p h d", d=DHEAD),
                            axis=AX.X, op=ALU.add)
                    nrm = smallp.tile([128, HEADS], F32, tag="nrm")
                    nc.scalar.activation(out=nrm, in_=ss, func=AF.Sqrt,
                                         bias=eps2_t, scale=1.0)
                    nc.vector.reciprocal(out=nrm, in_=nrm)
                    nrb = smallp.tile([128, HEADS], BF16, tag="nrb")
                    nc.vector.tensor_copy(out=nrb, in_=nrm)
                    nr = nrb[:, :]
                    nr_bc = bass.AP(tensor=nr.tensor, offset=nr.offset,
                                    ap=[nr.ap[0], [1, HEADS], [0, DHEAD]])
                    t3 = k_t[:, it, :].rearrange("p (h d) -> p h d", d=DHEAD)
                    nc.vector.tensor_mul(out=t3, in0=t3, in1=nr_bc)
                    nc.vector.tensor_mul(out=k_t[:, it, :], in0=k_t[:, it, :],
                                         in1=kg_bc)

            def attention(img, kT, qT, av_dst, filler=None):
                """block attention for one image; scores^T with the softmax
                denominator from v's ones-column."""
                i0 = img * TOK_IMG
                at_tiles = [None] * HEADS

                def emit_scores(hh):
                    c, b = hh // 2, (hh % 2) * DHEAD
                    sps = psp.tile([128, 2, TOK_IMG], F32, tag="sc")
                    for jc in range(2):
                        nc.tensor.matmul(
                            sps[:, jc, :],
                            kT[b:b + DHEAD, c,
                               i0 + jc * 128:i0 + (jc + 1) * 128],
                            qT[b:b + DHEAD, c, i0:i0 + TOK_IMG],
                            start=True, stop=True)
                    at = attnp.tile([128, 2, TOK_IMG], BF16, tag="at")
                    nc.scalar.activation(out=at, in_=sps, func=AF.Exp)
                    at_tiles[hh] = at

                def emit_av(hh):
                    at = at_tiles[hh]
                    aps = psp.tile([128, 2, DHEAD + 1], F32, tag="av")
                    for ic in range(2):
                        for jc in range(2):
                            nc.tensor.matmul(
                                aps[:, ic, :],
                                at[:, jc, ic * 128:(ic + 1) * 128],
                                v_t[:, 2 * img + jc, hh, :],
                                start=(jc == 0), stop=(jc == 1))
                    rs = smallp.tile([128, 2], F32, tag="rs")
                    nc.vector.reciprocal(
                        out=rs,
                        in_=aps[:, :, DHEAD:DHEAD + 1].rearrange(
                            "p a b -> p (a b)"))
                    rs_bc = bass.AP(tensor=rs.tensor, offset=rs.offset,
                                    ap=[rs.ap[0], [1, 2], [0, DHEAD]])
                    nc.vector.tensor_mul(
                        out=av_dst[:, 2 * img:2 * img + 2,
                                   hh * DHEAD:(hh + 1) * DHEAD],
                        in0=aps[:, :, 0:DHEAD], in1=rs_bc)

                for hh in range(HEADS):
                    emit_scores(hh)
                    if filler is not None:
                        f = next(filler, None)
                        if f is not None:
                            f()
                    if hh > 0:
                        emit_av(hh - 1)
                emit_av(HEADS - 1)

            def load_aux(dram, row, width, tag, dt=F32):
                t = auxp.tile([128, width], dt, tag=tag)
                nc.sync.dma_start(out=t, in_=bcast_ap(dram, row, width))
                return t

            # ---------------- embedding ----------------
            # patches go FIRST on the sync queue (they gate the very first
            # compute); weights follow; pos rides the gpsimd queue.
            psrc = patches_d.ap().rearrange("(t p) d -> p t d", p=128)
            pos_src = pos_d.ap().rearrange("(t p) d -> p t d", p=128)
            pts, poss = [], []
            for it in range(TT):
                pt = embp.tile([128, 1, PATCH_DIM], F32, tag="pt", bufs=4)
                nc.sync.dma_start(out=pt, in_=psrc[:, it:it + 1, :])
                pts.append(pt)
            wemb = load_w(W_emb_d.ap(), PATCH_DIM, DIM)
            g_emb2 = load_aux(emb_ln2_g_d, None, DIM, "g1")
            for it in range(TT):
                po = embp.tile([128, DIM], F32, tag="pos")
                nc.gpsimd.dma_start(out=po, in_=pos_src[:, it, :])
                poss.append(po)

            for it in range(TT):
                ln_to(xn_t, pts[it], [0], dst_it=it)
                transpose_tm(xnT_t, xn_t, [it])
            for it in range(TT):
                for g0 in range(0, DIM, 512):
                    gw = min(512, DIM - g0)
                    ps = psp.tile([128, 512], F32, tag="mm")
                    for c in range(KD):
                        nc.tensor.matmul(ps[:, :gw],
                                         xnT_t[:, c, it * 128:(it + 1) * 128],
                                         wemb[:, c, g0:g0 + gw],
                                         start=(c == 0), stop=(c == KD - 1))
                    nc.scalar.activation(out=x_t[:, it, g0:g0 + gw],
                                         in_=ps[:, :gw], func=AF.Copy)
                if use_bemb:
                    nc.gpsimd.dma_start(out=x_t[:, it, :],
                                        in_=bcast_ap(b_emb_d, None, DIM),
                                        accum_op=ALU.add)
                ln_to(x_t, x_t, [it], gamma_bc=g_emb2)
                nc.vector.tensor_add(out=x_t[:, it, :], in0=x_t[:, it, :],
                                     in1=poss[it])
                ln_to(xn_t, x_t, [it])
            for it in range(TT):
                transpose_tm(xnT_t, xn_t, [it])

            # ---------------- transformer layers ----------------
            # invariant at layer entry: xn_t holds LN_attn(x); xnT holds its
            # transpose for tiles 0,1 (all 4 at layer 0); tiles 2,3 are
            # transposed under this layer's first GEMMs.
            for l in range(DEPTH):
                wq = load_w(Wq_d[l], DIM, INNER)
                wkv = load_w(Wkv_d[l], DIM, 2 * INNER)
                kg_bc = load_aux(kg_row_d, l, INNER, "g2", BF16)
                b1t = auxp.tile([128, MC], F32, tag="b1")
                nc.sync.dma_start(out=b1t,
                                  in_=b1_d[l].rearrange("(c p) -> p c", p=128))
                wo = load_w(Wo_d[l], INNER, DIM)

                # img0 QKV; tail transposes of the previous layer ride along.
                # attention(0) is hoisted BEFORE img1's QKV so the serialized
                # rms->sqrt->exp-table->exp chain of each image runs under the
                # other image's GEMMs instead of on the critical path.
                mm_tok(q_t, xnT_t, wq, INNER, [0, 1])
                mm_tok(k_t, xnT_t, wkv, INNER, [0, 1])
                mm_v(xnT_t, wkv, INNER, [0, 1])
                rms_q([0, 1], kg_bc)
                rms_knorm([0, 1])
                if l > 0:
                    transpose_tm(xnT_t, xn_t, [2, 3])
                transpose_tm(qT_t, q_t, [0, 1], INNER)
                transpose_tm(kT_t, k_t, [0, 1], INNER)

                # img1's 12 QKV GEMM groups ride as fillers between
                # attention(0)'s heads, so img1's psums (and the whole
                # rms -> sqrt -> exp-table chain) complete progressively
                # during attention(0) instead of serializing after it
                def qkv1_thunks():
                    th = []
                    for dst, cb in ((q_t, 0), (k_t, 0), (v_t, INNER)):
                        for it in (2, 3):
                            for g0 in (0, 512):
                                def t(dst=dst, cb=cb, it=it, g0=g0):
                                    gw = min(512, INNER - g0)
                                    ps = psp.tile([128, 512], F32, tag="mm")
                                    for c in range(KD):
                                        nc.tensor.matmul(
                                            ps[:, :gw],
                                            xnT_t[:, c, it * 128:(it + 1) * 128],
                                            wkv[:, c, cb + g0:cb + g0 + gw]
                                            if dst is not q_t else
                                            wq[:, c, g0:g0 + gw],
                                            start=(c == 0), stop=(c == KD - 1))
                                    if dst is v_t:
                                        h0, nh = g0 // DHEAD, gw // DHEAD
                                        nc.scalar.activation(
                                            out=v_t[:, it, h0:h0 + nh, 0:DHEAD],
                                            in_=ps[:, :gw].rearrange(
                                                "p (h d) -> p h d", d=DHEAD),
                                            func=AF.Copy)
                                    else:
                                        nc.scalar.activation(
                                            out=dst[:, it, g0:g0 + gw],
                                            in_=ps[:, :gw], func=AF.Copy)
                                th.append(t)
                    return th

                attention(0, kT_t, qT_t, av_t, filler=iter(qkv1_thunks()))
                rms_q([2, 3], kg_bc)
                rms_knorm([2, 3])
                transpose_tm(avT_t, av_t, [0, 1], INNER)
                mm_tok(None, avT_t, wo, DIM, [0, 1], add_resid=x_t)
                transpose_tm(qT_t, q_t, [2, 3], INNER)
                transpose_tm(kT_t, k_t, [2, 3], INNER)
                attention(1, kT_t, qT_t, av_t)
                transpose_tm(avT_t, av_t, [2, 3], INNER)
                mm_tok(None, avT_t, wo, DIM, [2, 3], add_resid=x_t)
                # ln_ff after both images' exps -> the ACT sqrt burst doesn't
                # thrash the exp table mid-attention
                ln_to(xn_t, x_t, [0, 1, 2, 3])

                # ---- MLP: W1 bf16 (feature-major h, fp8 out), W2 fp8 DR ----
                # W1 runs per-image so image 0's GEMMs hide the tail LN and
                # transposes of tiles 2,3
                w1a = load_w(W1_d[l], DIM, MLP // 2, col_off=0)
                w1b = load_w(W1_d[l], DIM, MLP // 2, col_off=MLP // 2)

                def w1_chunk(half, w1h, mf, sl):
                    ps = psp.tile([128, T], F32, tag="mm")
                    for c in range(KD):
                        nc.tensor.matmul(
                            ps[:, sl], w1h[:, c, mf * 128:(mf + 1) * 128],
                            xnT_t[:, c, sl],
                            start=(c == 0), stop=(c == KD - 1))
                    nc.scalar.activation(
                        out=h_t[:, half * (MC // 2) + mf, sl],
                        in_=ps[:, sl], func=AF.Gelu,
                        bias=b1t[:, half * (MC // 2) + mf:
                                 half * (MC // 2) + mf + 1],
                        scale=1.0)

                # first few chunks run per-image so their GEMMs cover the
                # tail LN + transposes of tiles 2,3; the rest stream all
                # 512 tokens per matmul (N=512 beats 2x N=256)
                SPLIT = 3
                transpose_tm(xnT_t, xn_t, [0, 1])
                for mf in range(SPLIT):
                    w1_chunk(0, w1a, mf, slice(0, TOK_IMG))
                transpose_tm(xnT_t, xn_t, [2, 3])
                for mf in range(SPLIT):
                    w1_chunk(0, w1a, mf, slice(TOK_IMG, T))
                for mf in range(SPLIT, MC // 2):
                    w1_chunk(0, w1a, mf, slice(0, T))
                for mf in range(MC // 2):
                    w1_chunk(1, w1b, mf, slice(0, T))

                w2t = load_w(W2_d[l], MLP, DIM, dt=W8)
                if l == DEPTH - 1:
                    # prefetch the pool weights under the last MLP
                    pwkv = load_w(pWkv_d.ap(), DIM, 2 * INNER)
                    pk_bc = load_aux(pk_row_d, None, INNER, "g2", BF16)
                    qpool = constp.tile([128, HEADS], BF16)
                    nc.sync.dma_start(out=qpool, in_=qpool_d[:, :])

                def w2_tile(it):
                    psA = psp.tile([128, 512], F32, tag="sc")
                    psB = psp.tile([128, 256], F32, tag="av")
                    for m2 in range(MC // 2):
                        lhs = h_t[:, 2 * m2:2 * m2 + 2,
                                  it * 128:(it + 1) * 128]
                        nc.tensor.matmul(
                            psA, lhs, w2t[:, 2 * m2:2 * m2 + 2, 0:512],
                            start=(m2 == 0), stop=(m2 == MC // 2 - 1),
                            perf_mode=DR)
                        nc.tensor.matmul(
                            psB, lhs, w2t[:, 2 * m2:2 * m2 + 2, 512:768],
                            start=(m2 == 0), stop=(m2 == MC // 2 - 1),
                            perf_mode=DR)
                    nc.vector.scalar_tensor_tensor(
                        out=x_t[:, it, 0:512], in0=psA, scalar=W1_DEQ,
                        in1=x_t[:, it, 0:512], op0=ALU.mult, op1=ALU.add)
                    nc.vector.scalar_tensor_tensor(
                        out=x_t[:, it, 512:768], in0=psB, scalar=W1_DEQ,
                        in1=x_t[:, it, 512:768], op0=ALU.mult, op1=ALU.add)
                    if use_b2:
                        nc.gpsimd.dma_start(out=x_t[:, it, :],
                                            in_=bcast_ap(b2_d, l, DIM),
                                            accum_op=ALU.add)
                    ln_to(xn_t, x_t, [it])

                if l < DEPTH - 1:
                    # stagger: transposes of t0/t1 hide under W2 of t2/t3;
                    # t2/t3 transposes ride under the next layer's QKV.
                    w2_tile(0)
                    w2_tile(1)
                    transpose_tm(xnT_t, xn_t, [0])
                    w2_tile(2)
                    transpose_tm(xnT_t, xn_t, [1])
                    w2_tile(3)
                else:
                    for it in range(TT):
                        w2_tile(it)

            # ---------------- final LN + attention pooling ----------------
            # (final LN already in xn from the last layer's MLP tail; the
            # pool K/V GEMM per tile hides the LN/transpose chains; pwkv,
            # pk_bc and qpool were prefetched under layer 3's MLP)
            pwo = load_w(pWo_d.ap(), INNER, DIM)
            pq_bc = load_aux(pool_q_d, None, DIM, "g2")
            whead = load_w(W_head_d.ap(), DIM, NCLS)

            pooled = actp.tile([1, IMG_PER_CORE, DIM], F32, tag="pooled")

            def pool_kv_tile(it):
                transpose_tm(xnT_t, xn_t, [it])
                mm_tok(k_t, xnT_t, pwkv, INNER, [it])
                mm_v(xnT_t, pwkv, INNER, [it])
                rms_k_pool([it], pk_bc)

            def pool_attn(img):
                i0 = img * TOK_IMG
                # batched scores: one [128j, 2-head] matmul per (jc, chunk);
                # qpool has each head's 64 rows zero-padded to 128.
                sps = psp.tile([128, 2, KD, 2], F32, tag="sc")
                for jc in range(2):
                    for c in range(KD):
                        nc.tensor.matmul(
                            sps[:, jc, c, :],
                            kT_t[:, c, i0 + jc * 128:i0 + (jc + 1) * 128],
                            qpool[:, 2 * c:2 * c + 2],
                            start=True, stop=True)
                acl = attnp.tile([128, 2, KD, 2], BF16, tag="acl")
                nc.scalar.activation(out=acl, in_=sps, func=AF.Exp)
                for hh in range(HEADS):
                    c, h2 = hh // 2, hh % 2
                    aps = psp.tile([128, DHEAD + 1], F32, tag="av")
                    for jc in range(2):
                        nc.tensor.matmul(
                            aps[0:1, :],
                            acl[:, jc, c, h2:h2 + 1],
                            v_t[:, 2 * img + jc, hh, :],
                            start=(jc == 0), stop=(jc == 1))
                    rs = smallp.tile([1, 1], F32, tag="rsp")
                    nc.vector.reciprocal(out=rs, in_=aps[0:1, DHEAD:DHEAD + 1])
                    nc.vector.tensor_scalar_mul(
                        out=pooled[0:1, img, hh * DHEAD:(hh + 1) * DHEAD],
                        in0=aps[0:1, 0:DHEAD], scalar1=rs)

            # both images' pooled vectors live on partition 0 -> batch the
            # pWo GEMM, head-LN and W_head GEMM across images (out parts 0:2)
            # software-pipelined: each tile's kT transpose is emitted one
            # tile late so the next tile's K/V GEMMs cover its rms-chain wait
            pool_kv_tile(0)
            pool_kv_tile(1)
            transpose_tm(kT_t, k_t, [0], INNER)
            pool_kv_tile(2)
            transpose_tm(kT_t, k_t, [1], INNER)
            pool_attn(0)
            pool_kv_tile(3)
            transpose_tm(kT_t, k_t, [2], INNER)
            transpose_tm(kT_t, k_t, [3], INNER)
            pool_attn(1)

            pooled_bf = actp.tile([1, IMG_PER_CORE, DIM], BF16, tag="pooledb")
            nc.vector.tensor_copy(out=pooled_bf, in_=pooled)
            pT = actp.tile([128, KD, IMG_PER_CORE], BF16, tag="p2T")
            psT = psp.tile([128, KD, 2, 2], BF16, tag="tp")
            for c in range(KD):
                for img in range(IMG_PER_CORE):
                    nc.tensor.transpose(
                        psT[:, c, img, 0:1],
                        pooled_bf[0:1, img, c * 128:(c + 1) * 128],
                        ident[0:1, 0:1])
            nc.vector.tensor_copy(out=pT, in_=psT[:, :, :, 0:1])
            pool2 = actp.tile([IMG_PER_CORE, DIM], F32, tag="pool2")
            for g0 in range(0, DIM, 512):
                gw = min(512, DIM - g0)
                ps = psp.tile([128, 512], F32, tag="mm")
                for c in range(KD):
                    nc.tensor.matmul(ps[0:IMG_PER_CORE, :gw], pT[:, c, :],
                                     pwo[:, c, g0:g0 + gw],
                                     start=(c == 0), stop=(c == KD - 1))
                nc.vector.tensor_add(out=pool2[:, g0:g0 + gw],
                                     in0=ps[0:IMG_PER_CORE, :gw],
                                     in1=pq_bc[0:IMG_PER_CORE, g0:g0 + gw])

            st = smallp.tile([IMG_PER_CORE, 2, nc.vector.BN_STATS_DIM], F32,
                             tag="stp")
            pr = pool2.rearrange("p (n f) -> p n f", f=384)
            for i in range(2):
                nc.vector.bn_stats(out=st[:, i, :], in_=pr[:, i, :])
            mv = smallp.tile([IMG_PER_CORE, nc.vector.BN_AGGR_DIM], F32,
                             tag="mvp")
            nc.vector.bn_aggr(out=mv, in_=st)
            rstd = smallp.tile([IMG_PER_CORE, 1], F32, tag="rstdp")
            nc.scalar.activation(out=rstd, in_=mv[:, 1:2], func=AF.Sqrt,
                                 bias=eps_t[0:IMG_PER_CORE], scale=1.0)
            nc.vector.reciprocal(out=rstd, in_=rstd)
            hn = actp.tile([IMG_PER_CORE, DIM], BF16, tag="hn")
            nc.vector.tensor_scalar(out=hn, in0=pool2, scalar1=mv[:, 0:1],
                                    scalar2=rstd, op0=ALU.subtract,
                                    op1=ALU.mult)

            hT = actp.tile([128, KD, IMG_PER_CORE], BF16, tag="h2T")
            psT2 = psp.tile([128, KD, 2], BF16, tag="tp")
            for c in range(KD):
                nc.tensor.transpose(
                    psT2[:, c, :],
                    hn[0:IMG_PER_CORE, c * 128:(c + 1) * 128],
                    ident[0:IMG_PER_CORE, 0:IMG_PER_CORE])
            nc.vector.tensor_copy(out=hT, in_=psT2)
            out_sb = actp.tile([IMG_PER_CORE, NCLS], F32, tag="outsb")
            for g0 in range(0, NCLS, 500):
                gw = min(500, NCLS - g0)
                ps = psp.tile([128, 512], F32, tag="mm")
                for c in range(KD):
                    nc.tensor.matmul(ps[0:IMG_PER_CORE, :gw], hT[:, c, :],
                                     whead[:, c, g0:g0 + gw],
                                     start=(c == 0), stop=(c == KD - 1))
                nc.vector.tensor_copy(out=out_sb[:, g0:g0 + gw],
                                      in_=ps[0:IMG_PER_CORE, :gw])
            nc.sync.dma_start(out=out_d[0:IMG_PER_CORE, :], in_=out_sb)

    nc.finalize()
    return nc


def _prep_inputs(inp):
    bf = ml_dtypes.bfloat16
    f8 = ml_dtypes.float8_e4m3
    f32 = np.float32
    pp = np.asarray(inp["patch_positions"])
    pos_add = (inp["pos_h"][pp[..., 0]] + inp["pos_w"][pp[..., 1]]).astype(f32)

    kg = (inp["qn_g"].reshape(DEPTH, INNER) * inp["kn_g"].reshape(DEPTH, INNER)
          * SCALE * SCALE).astype(f32)
    pk = (inp["p_kn_g"].reshape(INNER) * SCALE).astype(f32)

    qn = _ln_np(inp["pool_q"].astype(f32), inp["pool_ln_g"]) @ inp["pWq"]
    qn = _rms_np(qn.reshape(HEADS, DHEAD), inp["p_qn_g"]).reshape(INNER)
    qpool = np.zeros((128, HEADS), dtype=bf)
    for h in range(HEADS):
        r0 = (h % 2) * DHEAD
        qpool[r0:r0 + DHEAD, h] = qn[h * DHEAD:(h + 1) * DHEAD].astype(bf)

    # fold LN gammas into the consuming weight matrices (gamma scales the
    # contraction rows)
    W_emb = (np.asarray(inp["emb_ln_g"])[:, None]
             * np.asarray(inp["W_emb"], f32))
    ln_attn_g = np.asarray(inp["ln_attn_g"], f32)
    Wq = ln_attn_g[:, :, None] * np.asarray(inp["Wq"], f32)
    Wkv = ln_attn_g[:, :, None] * np.asarray(inp["Wkv"], f32)
    W1 = np.asarray(inp["ln_ff_g"], f32)[:, :, None] * np.asarray(inp["W1"], f32)
    pWkv = np.asarray(inp["final_ln_g"], f32)[:, None] * np.asarray(inp["pWkv"], f32)
    W_head = np.asarray(inp["head_ln_g"], f32)[:, None] * np.asarray(inp["W_head"], f32)

    W2_q = np.clip(np.asarray(inp["W2"], f32) * W_SCALE, -240, 240).astype(f8)

    common = dict(
        W_emb=W_emb.astype(bf),
        Wq=Wq.astype(bf),
        Wkv=Wkv.astype(bf),
        Wo=np.asarray(inp["Wo"]).astype(bf),
        W1=W1.astype(bf),
        W2=W2_q,
        pWkv=pWkv.astype(bf),
        pWo=np.asarray(inp["pWo"]).astype(bf),
        W_head=W_head.astype(bf),
        emb_ln2_g=np.asarray(inp["emb_ln2_g"]).astype(f32),
        b_emb=np.asarray(inp["b_emb"]).astype(f32),
        kg_row=kg.astype(bf),
        b1=np.asarray(inp["b1"]).astype(f32),
        b2=np.asarray(inp["b2"]).astype(f32),
        pk_row=pk.astype(bf), qpool=qpool,
        pool_q=np.asarray(inp["pool_q"]).astype(f32),
    )
    patches = np.asarray(inp["patches"]).reshape(B * IMGS, TOK_IMG, PATCH_DIM)
    pos_add = pos_add.reshape(B * IMGS, TOK_IMG, DIM)
    in_maps = []
    for c in range(NCORES):
        m = dict(common)
        m["patches"] = np.ascontiguousarray(
            patches[c * 2:(c + 1) * 2].reshape(T, PATCH_DIM)).astype(f32)
        m["pos_add"] = np.ascontiguousarray(
            pos_add[c * 2:(c + 1) * 2].reshape(T, DIM)).astype(f32)
        in_maps.append(m)
    return in_maps


def _is_fast_path(inp):
    ids = np.asarray(inp["image_ids"])
    want = np.broadcast_to(np.repeat(np.arange(IMGS), PH * PW)[None], (B, N))
    return (ids.shape == (B, N) and np.array_equal(ids, want)
            and np.all(np.asarray(inp["lengths"]) == N))


def kernel(**inputs):
    inputs = {k: np.asarray(v) for k, v in inputs.items()}
    if not _is_fast_path(inputs):
        return _reference_np(**inputs)

    from concourse.bass_utils import run_bass_kernel_spmd

    cfg = dict(bemb=bool(np.any(inputs["b_emb"])),
               b2=bool(np.any(inputs["b2"])))
    key = ("nc", cfg["bemb"], cfg["b2"])
    if key not in _CACHE:
        _CACHE[key] = build_kernel(cfg)
        _CACHE["nc"] = _CACHE[key]
    nc = _CACHE[key]
    in_maps = _prep_inputs(inputs)
    res = run_bass_kernel_spmd(nc, in_maps, core_ids=list(range(NCORES)))
    out = np.stack([res.results[c]["out"] for c in range(NCORES)])
    return out.reshape(B, IMGS, NCLS).astype(np.float32)
